# revision 37
# baseline (speedup 1.0000x reference)
"""BEV histogram-binning + 4x(conv3x3+BN+ReLU) + 3x maxpool on 8 trn2 cores.

Sharding: data-parallel over (batch, y-half): core c = 2*b + h computes output
rows [64h, 64h+64) of batch b's final [64,128,128] block. Points are binned on
the host; the device runs the conv stack.

Design highlights (v10):
- L1 conv in fp8e4m3 DoubleRow matmuls (0.5 cyc/row): weights split hi+lo fp8
  (~f16 precision), BEV input plain fp8. Per 256-col block 3 DR instrs:
  (whi dx0|whi dx2) k-stride 2, (wlo dx0|wlo dx2) stride 2,
  (whi dx1|wlo dx1) stride 0 with fp8 bias hi/lo rows vs a ones-row (k=72).
- L2-4 f16 matmuls; bias via ones-rows (L2/L3) or ACT epilogue (L4).
- All conv matmuls write x-parity-DEINTERLEAVED psum (even x cols first, odd
  x cols second) so pooling reads contiguous slices.
- Epilogue per conv tile (psum [128, W], M = (y%2)*64 + (y//2)*co + o):
    tmp = ACT Relu copy of psum (L1: even half; L2/3: full) -> f16
    xp  = DVE TT max: L1 (tmp, ps-odds); L2/3 (tmp-evens, tmp-odds) 2x mode
    xo  = Pool TS copy xp[64:128] -> base 0
    dst = DVE TT max(xp[0:64], xo) -> f16 dst buffer (2x mode)
  SAME-pad y-edges via 7 per-core mask multiplies (ACT).
- Tile-to-tile halo rows duplicated by strided SBUF-SBUF DMAs batched 4 tiles
  per transfer (HWDGE-cheap); layer activations in single big SBUF buffers.
"""
import sys
sys.path.insert(0, '/opt/trn_rl_repo')
import numpy as np
import ml_dtypes

F8 = ml_dtypes.float8_e4m3fn

PR = [0.0, -39.68, -3.0, 69.12, 39.68, 1.0]
W = 1024
H = 1024
B = 4
BN_EPS = 1e-5

L2K, L3K, L4K = 81, 97, 128
NT1, NT2, NT3, NT4 = 34, 34, 33, 32
L2W, L3W, L4W = 514, 258, 130

# edge mask regions: (buf, tile, a0, a1, z0, z1, active_h)
# region [a0,a1) is multiplied by a mask that is 0 on [z0,z1) iff core half
# matches active_h; a0 must be 32-aligned (engine partition-access rule)
EDGES = [
    ("l2", 0, 0, 64, 0, 56, 0),
    ("l3", 0, 0, 64, 0, 48, 0),
    ("l4", 0, 0, 32, 0, 32, 0),
    ("l2", 32, 32, 64, 56, 64, 1),
    ("l2", 32, 64, 80, 64, 80, 1),
    ("l2", 33, 0, 80, 0, 80, 1),
    ("l3", 32, 32, 64, 48, 64, 1),
    ("l3", 32, 64, 96, 64, 96, 1),
    ("l4", 31, 96, 128, 96, 128, 1),
]

_CACHE = {}


def _bin_points(points):
    pts = np.asarray(points, dtype=np.float32)
    xs = np.float32(W / (PR[3] - PR[0]))
    ys = np.float32(H / (PR[4] - PR[1]))
    half = np.float32((PR[4] - PR[1]) / 2)
    xp = (pts[:, 1] * xs).astype(np.int32)
    yp = ((pts[:, 2] + half) * ys).astype(np.int32)
    b = pts[:, 0].astype(np.int32)
    mask = (xp >= 0) & (xp < W) & (yp >= 0) & (yp < H)
    lin = (b * H + yp) * W + xp
    z = pts[:, 3]
    inten = pts[:, 4]
    n = B * H * W
    lv = lin[mask]
    cnt = np.bincount(lv, minlength=n).astype(np.float32)
    zmin = np.full(n, 10.0, np.float32)
    np.minimum.at(zmin, lv, z[mask])
    zmax = np.full(n, -10.0, np.float32)
    np.maximum.at(zmax, lv, z[mask])
    iv = np.zeros(n, np.float32)
    np.maximum.at(iv, lv, inten[mask])
    bev0 = np.where(cnt == 0, np.float32(1.0), cnt) / np.float32(50.0)
    grids = np.stack([bev0, zmin, zmax, iv], axis=0).reshape(4, B, H, W)
    return np.transpose(grids, (1, 0, 2, 3))


def _fold_weights(w, b, g, be, m, v):
    scale = np.asarray(g, np.float32) / np.sqrt(np.asarray(v, np.float32) + np.float32(BN_EPS))
    wf = np.asarray(w, np.float32) * scale[:, None, None, None]
    bf = (np.asarray(b, np.float32) - np.asarray(m, np.float32)) * scale + np.asarray(be, np.float32)
    return wf, bf


def _banded(wdx, ci, co, yoff, K):
    out = np.zeros((K, 128), np.float32)
    for y in range(yoff):
        m0 = (y % 2) * 64 + (y // 2) * co
        for dy in range(3):
            e = y + dy
            out[e * ci:(e + 1) * ci, m0:m0 + co] = wdx[dy].T
    return out


def _banded_l4(wdx, ci, co, yoff, K):
    out = np.zeros((K, 128), np.float32)
    for y in range(yoff):
        for dy in range(3):
            e = y + dy
            for o in range(co):
                out[e * ci:(e + 1) * ci, o * yoff + y] = wdx[dy][o]
    return out


def _build_module():
    import concourse.mybir as mybir
    from concourse.tile import TileContext
    from concourse import bacc

    f8 = mybir.dt.float8e4
    f16 = mybir.dt.float16
    f32 = mybir.dt.float32
    AL = mybir.AluOpType
    DR = mybir.MatmulPerfMode.DoubleRow
    COPY = mybir.ActivationFunctionType.Copy
    RELU = mybir.ActivationFunctionType.Relu

    nc = bacc.Bacc()
    bev_d = nc.dram_tensor("bev", [73, NT1 * 1026], f8, kind="ExternalInput")
    wdr_d = nc.dram_tensor("wdr", [73, 768], f8, kind="ExternalInput")
    wl2_d = nc.dram_tensor("wl2", [L2K, 384], f16, kind="ExternalInput")
    wl3_d = nc.dram_tensor("wl3", [L3K, 384], f16, kind="ExternalInput")
    wl4_d = nc.dram_tensor("wl4", [L4K, 384], f16, kind="ExternalInput")
    bias4_d = nc.dram_tensor("bias4", [128, 1], f32, kind="ExternalInput")
    ones_d = nc.dram_tensor("ones", [1, NT2 * L2W], f16, kind="ExternalInput")
    edge_d = nc.dram_tensor("edge", [128, len(EDGES)], f32, kind="ExternalInput")
    zpad_d = nc.dram_tensor("zpad", [128, 80], f16, kind="ExternalInput")
    out_d = nc.dram_tensor("out", [128, 8 * 512], f16, kind="ExternalOutput")

    with TileContext(nc) as tc:
        with tc.tile_pool(name="const", bufs=1) as cpool, \
             tc.tile_pool(name="bufs", bufs=1) as bpool, \
             tc.tile_pool(name="work", bufs=4) as wp, \
             tc.tile_pool(name="psum", bufs=1, space="PSUM") as pp:

            wdrt = cpool.tile([73, 768], f8, tag="wdrt")
            nc.sync.dma_start(out=wdrt[:], in_=wdr_d[:])
            wdr = [wdrt[:, 256 * i:256 * (i + 1)] for i in range(3)]
            bev = bpool.tile([73, NT1 * 1026], f8, tag="bev")
            # first small chunk so L1 can start early; rest after weights
            CHUNKS = [(0, 6), (6, 16), (16, 25), (25, 34)]
            t0, t1 = CHUNKS[0]
            nc.sync.dma_start(out=bev[:, t0 * 1026:t1 * 1026],
                              in_=bev_d[:, t0 * 1026:t1 * 1026])
            wl2 = cpool.tile([L2K, 384], f16, tag="wl2")
            wl3 = cpool.tile([L3K, 384], f16, tag="wl3")
            wl4 = cpool.tile([L4K, 384], f16, tag="wl4")
            bias4 = cpool.tile([128, 1], f32, tag="bias4")
            edge = cpool.tile([128, len(EDGES)], f32, tag="edge")
            nc.sync.dma_start(out=wl2[:], in_=wl2_d[:])
            nc.sync.dma_start(out=wl3[:], in_=wl3_d[:])
            nc.sync.dma_start(out=wl4[:], in_=wl4_d[:])
            nc.sync.dma_start(out=bias4[:], in_=bias4_d[:])
            nc.sync.dma_start(out=edge[:], in_=edge_d[:])
            for t0, t1 in CHUNKS[1:]:
                nc.sync.dma_start(out=bev[:, t0 * 1026:t1 * 1026],
                                  in_=bev_d[:, t0 * 1026:t1 * 1026])
            l2b = bpool.tile([L2K, NT2 * L2W], f16, tag="l2b")
            l3b = bpool.tile([L3K, NT3 * L3W], f16, tag="l3b")
            l4b = bpool.tile([L4K, NT4 * L4W], f16, tag="l4b")
            nc.sync.dma_start(out=l2b[80:81, :], in_=ones_d[0:1, :])
            nc.sync.dma_start(out=l3b[96:97, 0:NT3 * L3W], in_=ones_d[0:1, 0:NT3 * L3W])

            zpad = cpool.tile([128, 80], f16, tag="zpad")
            nc.sync.dma_start(out=zpad[:], in_=zpad_d[:])

            def pads(buf, K_, tw, nt):
                nc.gpsimd.memset(buf[0:K_, 0:1], 0.0)
                q = buf[0:K_, 0:2].copy()
                v = q.ap
                v.pop()
                v.append([tw, nt - 1]); v.append([1, 2])
                q.offset = q.offset + tw - 1
                nc.sync.dma_start(out=q, in_=zpad[0:K_, 0:2 * (nt - 1)])
                nc.gpsimd.memset(buf[0:K_, nt * tw - 1:nt * tw], 0.0)
            pads(l2b, 80, L2W, NT2)
            pads(l3b, 96, L3W, NT3)
            pads(l4b, 128, L4W, NT4)
            nc.gpsimd.memset(l2b[64:80, 33 * L2W:34 * L2W], 0.0)

            ps1 = [pp.tile([128, 1024], f32, tag=f"ps1_{i}", name=f"ps1_{i}")
                   for i in range(2)]
            ps2 = [pp.tile([128, 512], f32, tag=f"ps2_{i}", name=f"ps2_{i}")
                   for i in range(2)]
            ps3 = pp.tile([128, 512], f32, tag="ps3")
            ps4 = pp.tile([128, 512], f32, tag="ps4")

            def edge_op(i):
                (bufn, t, a0, a1, _, _, _) = EDGES[i]
                buf, tw = {"l2": (l2b, L2W), "l3": (l3b, L3W), "l4": (l4b, L4W)}[bufn]
                sl = buf[a0:a1, t * tw + 1:t * tw + tw - 1]
                nc.scalar.activation(out=sl, in_=sl, func=COPY,
                                     scale=edge[a0:a1, i:i + 1])

            def deint_out(ps, n_half, q, off):
                # out AP: evens -> [off, off+q), odds -> [n_half+off, ...)
                o = ps[:].copy()
                v = o.ap
                v.pop()
                v.append([1, q]); v.append([n_half, 2])
                o.offset = o.offset + off
                return o

            # ---------- L1 ----------
            def l1_mm(t):
                ps = ps1[t % 2]
                for b in range(4):
                    n0 = b * 256
                    # deinterleave within each 512-col half (one psum bank)
                    out = deint_out(ps, 256, 128, 512 * (b // 2) + (b % 2) * 128)
                    base = t * 1026 + n0
                    for i, (boff, stride) in enumerate([(0, 2), (0, 2), (1, 0)]):
                        rhs = bev[:, :].unsqueeze(1).copy()
                        v = rhs.ap
                        v.pop(); v.pop()
                        v.append([stride, 2]); v.append([1, 256])
                        rhs.offset = rhs.offset + base + boff
                        nc.tensor.matmul(out=out,
                                         lhsT=wdr[i].rearrange("p (two m) -> p two m", two=2),
                                         rhs=rhs, start=(i == 0), stop=(i == 2),
                                         perf_mode=DR)

            def l1_ep(t):
                ps = ps1[t % 2]
                evn = ps[:, 0:256].copy()
                v = evn.ap
                v.pop()
                v.append([512, 2]); v.append([1, 256])
                odd = evn.copy()
                odd.offset = odd.offset + 256
                tmp = wp.tile([128, 512], f16, tag="tmp1", name="tmp1")
                nc.scalar.activation(out=tmp[:], in_=evn, func=RELU,
                                     bias=0.0, scale=1.0)
                xp = wp.tile([128, 512], f16, tag="xp1", name="xp1", bufs=3)
                nc.vector.tensor_tensor(out=xp[:], in0=tmp[:], in1=odd,
                                        op=AL.max)
                xo = wp.tile([64, 512], f16, tag="xo1", name="xo1")
                nc.gpsimd.tensor_scalar(out=xo[:], in0=xp[64:128],
                                        scalar1=0.0, scalar2=None, op0=AL.add)
                c0 = t * L2W + 1
                nc.vector.tensor_tensor(out=l2b[0:64, c0:c0 + 512],
                                        in0=xp[0:64], in1=xo[:], op=AL.max)

            def halo_dma(buf, src_p0, src_p1, dst_p0, tw, t_src0, n, width,
                         eng=None):
                src = buf[src_p0:src_p1, 0:width].copy()
                v = src.ap
                v.pop()
                v.append([tw, n]); v.append([1, width])
                src.offset = src.offset + t_src0 * tw + 1
                dst = buf[dst_p0:dst_p0 + (src_p1 - src_p0), 0:width].copy()
                v = dst.ap
                v.pop()
                v.append([tw, n]); v.append([1, width])
                dst.offset = dst.offset + (t_src0 - 1) * tw + 1
                (eng or nc.sync).dma_start(out=dst, in_=src)

            # ---------- L2 ----------
            def l2_mm(u):
                ps = ps2[u % 2]
                out = deint_out(ps, 256, 256, 0)
                for dx in range(3):
                    nc.tensor.matmul(out=out,
                                     lhsT=wl2[:, dx * 128:(dx + 1) * 128],
                                     rhs=l2b[0:L2K, u * L2W + dx:u * L2W + dx + 512],
                                     start=(dx == 0), stop=(dx == 2))

            def l2_ep(u):
                ps = ps2[u % 2]
                tmp = wp.tile([128, 512], f16, tag="tmp2", name="tmp2")
                nc.scalar.activation(out=tmp[:], in_=ps[:], func=RELU,
                                     bias=0.0, scale=1.0)
                xp = wp.tile([128, 256], f16, tag="xp2", name="xp2")
                nc.vector.tensor_tensor(out=xp[:], in0=tmp[:, 0:256],
                                        in1=tmp[:, 256:512], op=AL.max)
                xo = wp.tile([64, 256], f16, tag="xo2", name="xo2")
                nc.gpsimd.tensor_scalar(out=xo[:], in0=xp[64:128],
                                        scalar1=0.0, scalar2=None, op0=AL.add)
                if u < 33:
                    c0 = u * L3W + 1
                    nc.vector.tensor_tensor(out=l3b[0:64, c0:c0 + 256],
                                            in0=xp[0:64], in1=xo[:], op=AL.max)
                    if u >= 1:
                        cp = (u - 1) * L3W + 1
                        nc.vector.tensor_tensor(out=l3b[64:96, cp:cp + 256],
                                                in0=xp[0:32], in1=xo[0:32],
                                                op=AL.max)
                else:
                    cp = 32 * L3W + 1
                    nc.vector.tensor_tensor(out=l3b[64:96, cp:cp + 256],
                                            in0=xp[0:32], in1=xo[0:32], op=AL.max)

            # ---------- L3 (pairs) ----------
            def l3_mm(v, n):
                for j in range(v, v + n):
                    out = deint_out(ps3, 128, 128, (j - v) * 256)
                    for dx in range(3):
                        nc.tensor.matmul(
                            out=out,
                            lhsT=wl3[:, dx * 128:(dx + 1) * 128],
                            rhs=l3b[0:L3K, j * L3W + dx:j * L3W + dx + 256],
                            start=(dx == 0), stop=(dx == 2))

            def l3_ep(v, n):
                w2 = 256 * n
                tmp = wp.tile([128, 512], f16, tag="tmp3", name="tmp3")
                nc.scalar.activation(out=tmp[0:128, 0:w2], in_=ps3[:, 0:w2],
                                     func=RELU, bias=0.0, scale=1.0)
                xp = wp.tile([128, 256], f16, tag="xp3", name="xp3")
                ine = tmp[0:128, 0:w2].copy()
                v_ = ine.ap
                v_.pop()
                v_.append([256, n]); v_.append([1, 128])
                ino = ine.copy()
                ino.offset = ino.offset + 128
                nc.vector.tensor_tensor(out=xp[0:128, 0:128 * n], in0=ine, in1=ino,
                                        op=AL.max)
                xo = wp.tile([64, 256], f16, tag="xo3", name="xo3")
                nc.gpsimd.tensor_scalar(out=xo[0:64, 0:128 * n], in0=xp[64:128, 0:128 * n],
                                        scalar1=0.0, scalar2=None, op0=AL.add)
                # main writes (strided over n tiles; skip tile 32)
                nmain = min(v + n, NT4) - v
                if nmain > 0:
                    dsto = l4b[0:64, 0:128].copy()
                    v_ = dsto.ap
                    v_.pop()
                    v_.append([L4W, nmain]); v_.append([1, 128])
                    dsto.offset = dsto.offset + v * L4W + 1
                    nc.vector.tensor_tensor(out=dsto, in0=xp[0:64, 0:128 * nmain],
                                            in1=xo[0:64, 0:128 * nmain], op=AL.max)
                for j in range(v, v + n):
                    if j < 1 or j > NT4 - 1:
                        continue
                    o = (j - v) * 128
                    cp = (j - 1) * L4W + 1
                    nc.vector.tensor_tensor(out=l4b[64:128, cp:cp + 128],
                                            in0=xp[0:64, o:o + 128],
                                            in1=xo[0:64, o:o + 128], op=AL.max)
                if v + n > NT4:
                    # tile 32: direct halo into tile 31 tail
                    o = (NT4 - v) * 128
                    cp = (NT4 - 1) * L4W + 1
                    nc.vector.tensor_tensor(out=l4b[64:128, cp:cp + 128],
                                            in0=xp[0:64, o:o + 128],
                                            in1=xo[0:64, o:o + 128], op=AL.max)

            # ---------- L4 (pair groups, double-buffered halves of ps4) ----
            def l4_mm(w):
                g2 = (w // 2) % 2          # which half of ps4
                off = g2 * 256 + (w % 2) * 128
                for dx in range(3):
                    nc.tensor.matmul(out=ps4[:, off:off + 128],
                                     lhsT=wl4[:, dx * 128:(dx + 1) * 128],
                                     rhs=l4b[0:L4K, w * L4W + dx:w * L4W + dx + 128],
                                     start=(dx == 0), stop=(dx == 2))

            def l4_ep(g):
                g2 = g % 2
                st = wp.tile([128, 256], f16, tag="st4", name="st4")
                nc.scalar.activation(out=st[:], in_=ps4[:, g2 * 256:g2 * 256 + 256],
                                     func=RELU, bias=bias4[:], scale=1.0)
                nc.sync.dma_start(out=out_d[:, g * 256:(g + 1) * 256], in_=st[:])

            # ---------- software pipeline (same-step epilogues) ----------
            # NOTE: lags must keep every producer in a strictly earlier step
            # than its consumer given deep-first emission (halo(v) of layer n
            # is written by layer (n-1) ep of tile v+2); 4/6/8 races.
            # lag safety under deep-first emission: LAG3 >= LAG2+3,
            # LAG4 >= LAG3+2 (producers must land in strictly earlier steps)
            LAG2, LAG3, LAG4 = 4, 7, 9
            for i in range(NT4 + LAG4 + 6):
                # deeper layers first: PE is in-order, so a stalled L1 matmul
                # must not head-of-line block ready L2-L4 matmuls
                w = i - LAG4
                if 0 <= w < NT4:
                    l4_mm(w)
                    if w % 2 == 1:
                        l4_ep(w // 2)
                v = i - LAG3
                if 0 <= v < NT3 and v % 2 == 0:
                    l3_mm(v, 2 if v + 1 < NT3 else 1)
                    l3_ep(v, 2 if v + 1 < NT3 else 1)
                    if v == 0:
                        edge_op(2)
                    if v == 32:
                        edge_op(8)
                u = i - LAG2
                if 0 <= u < NT2:
                    l2_mm(u)
                    l2_ep(u)
                    if u == 0:
                        edge_op(1)
                    if u == 33:
                        edge_op(6)
                        edge_op(7)
                if i < NT1:
                    l1_mm(i)
                    l1_ep(i)
                    if i == 0:
                        edge_op(0)
                # l2 halo batch k: dst{2k+1, 2k+2} -> halos {2k, 2k+1}
                if i >= 3 and (i - 3) % 2 == 0 and (i - 3) // 2 < 16:
                    k = (i - 3) // 2
                    halo_dma(l2b, 0, 16, 64, L2W, 2 * k + 1, 2, 512)
                if i == 34:
                    halo_dma(l2b, 0, 16, 64, L2W, 33, 1, 512)
                    edge_op(3)
                    edge_op(4)
                    edge_op(5)

    nc.finalize()
    return nc


def _build_bev(grid_b, h):
    from numpy.lib.stride_tricks import sliding_window_view
    g0 = 512 * h - 15
    padded = np.zeros((4, 546, 1026), np.float32)
    lo = max(0, g0)
    hi = min(1024, g0 + 546)
    padded[:, lo - g0:hi - g0, 1:1025] = grid_b[:, lo:hi, :]
    wins = sliding_window_view(padded, 18, axis=1)
    wins = wins[:, 0:16 * NT1:16]
    tiles = np.transpose(wins, (1, 3, 0, 2))   # [34, 18, 4, 1026]
    out = np.empty((73, NT1 * 1026), F8)
    out[0:72, :] = np.ascontiguousarray(tiles).reshape(NT1, 72, 1026) \
        .transpose(1, 0, 2).reshape(72, NT1 * 1026).astype(F8)
    out[72, :] = np.float32(1.0)
    return out


def _host_weights(params):
    wf1, bf1 = _fold_weights(*params[0])
    wf2, bf2 = _fold_weights(*params[1])
    wf3, bf3 = _fold_weights(*params[2])
    wf4, bf4 = _fold_weights(*params[3])

    whi = wf1.astype(F8).astype(np.float32)
    wlo = (wf1 - whi).astype(F8).astype(np.float32)
    bhi = bf1.astype(F8).astype(np.float32)
    blo = (bf1 - bhi).astype(F8).astype(np.float32)

    def band1(wsrc, dx):
        return _banded([wsrc[:, :, dy, dx] for dy in range(3)], 4, 8, 16, 73)

    pairs = [(band1(whi, 0), band1(whi, 2)),
             (band1(wlo, 0), band1(wlo, 2)),
             (band1(whi, 1), band1(wlo, 1))]
    mo = np.zeros(128, np.int32)
    for y in range(16):
        for o in range(8):
            mo[(y % 2) * 64 + (y // 2) * 8 + o] = o
    pairs[2][0][72, :] = bhi[mo]
    pairs[2][1][72, :] = blo[mo]
    wdr = []
    for kt0, kt1 in pairs:
        wdr.append(np.stack([kt0, kt1], axis=1).reshape(73, 256).astype(F8))

    def bandl(wf, ci, co, yoff, K, l4=False):
        out = np.zeros((K, 3, 128), np.float32)
        fn = _banded_l4 if l4 else _banded
        for dx in range(3):
            out[:, dx, :] = fn([wf[:, :, dy, dx] for dy in range(3)], ci, co, yoff, K)
        return out

    wl2 = bandl(wf2, 8, 16, 8, L2K)
    mo2 = np.zeros(128, np.int32)
    for y in range(8):
        for o in range(16):
            mo2[(y % 2) * 64 + (y // 2) * 16 + o] = o
    wl2[80, 1, :] = bf2[mo2]
    wl3 = bandl(wf3, 16, 32, 4, L3K)
    mo3 = np.zeros(128, np.int32)
    for y in range(4):
        for o in range(32):
            mo3[(y % 2) * 64 + (y // 2) * 32 + o] = o
    wl3[96, 1, :] = bf3[mo3]
    wl4 = bandl(wf4, 32, 64, 2, L4K, l4=True)
    mo4 = np.repeat(np.arange(64), 2)
    bias4 = bf4[mo4].astype(np.float32).reshape(128, 1)

    return {
        "wdr": np.concatenate(wdr, axis=1),
        "wl2": wl2.reshape(L2K, 384).astype(np.float16),
        "wl3": wl3.reshape(L3K, 384).astype(np.float16),
        "wl4": wl4.reshape(L4K, 384).astype(np.float16),
        "bias4": bias4,
    }


def kernel(points, batch_size,
           w1, b1, g1, be1, m1, v1,
           w2, b2, g2, be2, m2, v2,
           w3, b3, g3, be3, m3, v3,
           w4, b4, g4, be4, m4, v4, **_):
    from concourse.bass_utils import run_bass_kernel_spmd

    grids = _bin_points(points)
    params = [(w1, b1, g1, be1, m1, v1), (w2, b2, g2, be2, m2, v2),
              (w3, b3, g3, be3, m3, v3), (w4, b4, g4, be4, m4, v4)]
    wts = _host_weights(params)
    ones = np.ones((1, NT2 * L2W), np.float16)

    core_ids = list(range(8))
    in_maps = []
    for core in core_ids:
        b, h = core // 2, core % 2
        edge_arr = np.ones((128, len(EDGES)), np.float32)
        for i, (_, _, a0, a1, z0, z1, eh) in enumerate(EDGES):
            if eh == h:
                edge_arr[z0:z1, i] = 0.0
        im = {"bev": _build_bev(grids[b], h),
              "edge": edge_arr, "ones": ones,
              "wl2": wts["wl2"], "wl3": wts["wl3"], "wl4": wts["wl4"],
              "bias4": wts["bias4"], "wdr": wts["wdr"],
              "zpad": np.zeros((128, 80), np.float16)}
        in_maps.append(im)

    if "nc" not in _CACHE:
        _CACHE["nc"] = _build_module()
    nc = _CACHE["nc"]

    r = run_bass_kernel_spmd(nc, in_maps, core_ids=core_ids)

    out_full = np.zeros((B, 64, 128, 128), np.float32)
    for i, core in enumerate(core_ids):
        b, h = core // 2, core % 2
        o = r.results[i]["out"].astype(np.float32)
        arr = o.reshape(64, 2, 16, 2, 128)
        arr = arr.transpose(0, 2, 3, 1, 4).reshape(64, 64, 128)
        out_full[b, :, 64 * h:64 * h + 64, :] = arr
    return out_full
